# revision 18
# baseline (speedup 1.0000x reference)
"""Butterfly (Givens) rotation network on TRN2, 8 NeuronCores.

Algorithm
---------
x: (8192, 4096) f32. 12 butterfly layers; layer l rotates pairs of features
differing in bit l of the feature index. Split into two linear stages:

  Stage A = layers 0-6: features mix only within 128-wide blocks b (bits 0-6)
            -> per-block 128x128 matrix A_b.
  Stage B = layers 7-11: features mix only across blocks at fixed within-block
            position p (bits 7-11) -> per-p 32x32 matrix B_p; grouping 4
            consecutive p per 128-partition tile gives block-diag 128x128.

Per 128-row tile (rows on partitions), all on the TensorEngine:
  pass1: per block b: PE-transpose x_b -> xT_b [f',r] (PSUM->SBUF copy),
         MM out[r,fo] = sum_f' xT_b[f',r] * A_bT[f',fo]  (lhsT=xT_b, rhs=A_bT)
         scatter-copy PSUM->SBUF into Y with f~ = (p//4)*128 + (p%4)*32 + b.
  pass2: per f~-tile t: PE-transpose Y_t -> z [f~',r],
         MM out[r,n] = sum z[f~',r] * BDT_t[f~',n], scatter-copy to natural
         feature order, DMA out.

Sharding: data-parallel over rows, 1024 rows/core; matrices replicated.
"""

import math
import numpy as np

DIM = 4096
NL = 12
NB = 32          # 128-wide feature blocks
ROWS = 8192
NCORES = 8
RPC = ROWS // NCORES     # rows per core
NT = RPC // 128          # 128-row tiles per core


# ---------------------------------------------------------------- host math

def _butterfly_np(x, angles):
    """float64 numpy copy of the reference butterfly."""
    x = np.asarray(x, np.float64)
    angles = np.asarray(angles, np.float64)
    B, d = x.shape
    for l in range(angles.shape[0]):
        stride = 2 ** l
        nblocks = d // (2 * stride)
        xr = x.reshape(B, nblocks, 2, stride)
        c = np.cos(angles[l]).reshape(nblocks, stride)
        s = np.sin(angles[l]).reshape(nblocks, stride)
        xi = xr[:, :, 0, :].copy()
        xj = xr[:, :, 1, :].copy()
        x = np.stack([c * xi + s * xj, -s * xi + c * xj], axis=2).reshape(B, d)
    return x


def _build_mats(angles):
    """Returns (amats, bmats) each [128, 4096] f32 in SBUF-ready layout."""
    angles = np.asarray(angles, np.float64)
    amats = np.zeros((128, DIM), np.float64)
    for b in range(NB):
        # A_bT[f_in, f_out]: butterfly of identity rows = F^T for this block
        amats[:, 128 * b:128 * b + 128] = _butterfly_np(
            np.eye(128), angles[0:7, 64 * b:64 * b + 64])
    bmats = np.zeros((128, DIM), np.float64)
    for t in range(32):
        for pl in range(4):
            p = 4 * t + pl
            BpT = _butterfly_np(np.eye(32), angles[7:12, p::128])
            bmats[32 * pl:32 * pl + 32, 128 * t + 32 * pl:128 * t + 32 * pl + 32] = BpT
    return amats.astype(np.float32), bmats.astype(np.float32)


def _build_bmI(angles):
    """Stage-B matrix for v3's tiling: tile c0 holds p in {c0, c0+32, c0+64,
    c0+96} at partitions q~ = 4t + i (t = block, i = p//32), an interleaved
    block-diagonal of the four 32x32 B_p matrices."""
    angles = np.asarray(angles, np.float64)
    bmI = np.zeros((128, DIM), np.float64)
    for c0 in range(32):
        for i in range(4):
            p = c0 + 32 * i
            BpT = _butterfly_np(np.eye(32), angles[7:12, p::128])
            rows = 4 * np.arange(32) + i
            cols = 128 * c0 + 4 * np.arange(32) + i
            bmI[np.ix_(rows, cols)] = BpT
    return bmI.astype(np.float32)


# ---------------------------------------------------------------- bass kernel

def _emit_kernel(ctx, tc, out, x, amats, bmats, ident):
    import concourse.bass as bass
    import concourse.mybir as mybir

    nc = tc.nc
    f32 = mybir.dt.float32

    consts = ctx.enter_context(tc.tile_pool(name="consts", bufs=1))
    xin = ctx.enter_context(tc.tile_pool(name="xin", bufs=3))
    ystage = ctx.enter_context(tc.tile_pool(name="ystage", bufs=3))
    ostage = ctx.enter_context(tc.tile_pool(name="ostage", bufs=3))
    sbst = ctx.enter_context(tc.tile_pool(name="sbst", bufs=6))
    psA = ctx.enter_context(tc.tile_pool(name="psA", bufs=4, space="PSUM"))
    psB = ctx.enter_context(tc.tile_pool(name="psB", bufs=4, space="PSUM"))

    am = consts.tile([128, DIM], f32, tag="amats")
    bm = consts.tile([128, DIM], f32, tag="bmats")
    idt = consts.tile([128, 128], f32, tag="ident")
    nc.sync.dma_start(idt[:], ident[:])

    # Greedy least-loaded assignment of PSUM->SBUF copies to DVE/ACT,
    # using measured per-copy costs (ns) for [128,512] fp32 from PSUM.
    load = {"dve": 0.0, "act": 0.0}
    cost = {("dve", "plain"): 685, ("dve", "scatter"): 700,
            ("act", "plain"): 570, ("act", "scatter"): 1127}

    def copy(dst, src, kind="plain"):
        eng = min(("dve", "act"), key=lambda e: load[e] + cost[(e, kind)])
        load[eng] += cost[(eng, kind)]
        (nc.vector.tensor_copy if eng == "dve" else nc.scalar.copy)(dst, src)

    for i in range(NT):
        xt = xin.tile([128, DIM], f32, tag="xt")
        if i == 0:
            # first tile: fine-grained x/amats chunk interleave so the very
            # first transposes and stage-A matmuls start as early as possible
            for c in range(8):
                nc.sync.dma_start(xt[:, 512 * c:512 * (c + 1)],
                                  x[0:128, 512 * c:512 * (c + 1)])
                nc.sync.dma_start(am[:, 512 * c:512 * (c + 1)],
                                  amats[:, 512 * c:512 * (c + 1)])
        else:
            nc.sync.dma_start(xt[:], x[128 * i:128 * (i + 1), :])
        Y = ystage.tile([128, DIM], f32, tag="Y")

        for g in range(8):           # groups of 4 feature blocks
            pt = psA.tile([128, 512], f32, tag="ptA")
            for j in range(4):
                b = 4 * g + j
                nc.tensor.transpose(
                    pt[:, 128 * j:128 * (j + 1)],
                    xt[:, 128 * b:128 * (b + 1)], idt[:])
            xT4 = sbst.tile([128, 512], f32, tag="xT4")
            copy(xT4[:], pt[:])
            pm = psB.tile([128, 512], f32, tag="pmA")
            for j in range(4):
                b = 4 * g + j
                nc.tensor.matmul(
                    pm[:, 128 * j:128 * (j + 1)],
                    xT4[:, 128 * j:128 * (j + 1)],
                    am[:, 128 * b:128 * (b + 1)],
                    start=True, stop=True)
            # scatter into Y: dest f~ = t*128 + pl*32 + (4g+j), src = j*128 + 4t + pl
            src = pm[:].rearrange("r (j t pl) -> r j t pl", j=4, t=32, pl=4)
            dst = Y[:].rearrange(
                "r (t pl g j) -> r g j t pl", t=32, pl=4, g=8, j=4)[:, g]
            copy(dst, src, kind="scatter")

        if i == 0:
            for c in range(8):
                nc.sync.dma_start(bm[:, 512 * c:512 * (c + 1)],
                                  bmats[:, 512 * c:512 * (c + 1)])
        O = ostage.tile([128, DIM], f32, tag="O")
        for g in range(8):           # groups of 4 f~ tiles
            pt = psA.tile([128, 512], f32, tag="ptA")
            for j in range(4):
                t = 4 * g + j
                nc.tensor.transpose(
                    pt[:, 128 * j:128 * (j + 1)],
                    Y[:, 128 * t:128 * (t + 1)], idt[:])
            z4 = sbst.tile([128, 512], f32, tag="xT4")
            copy(z4[:], pt[:])
            pm = psB.tile([128, 512], f32, tag="pmA")
            for j in range(4):
                t = 4 * g + j
                nc.tensor.matmul(
                    pm[:, 128 * j:128 * (j + 1)],
                    z4[:, 128 * j:128 * (j + 1)],
                    bm[:, 128 * t:128 * (t + 1)],
                    start=True, stop=True)
            # scatter to natural order: dest f = b*128 + 4t + pl = b*128 + 16g + 4j + pl
            src = pm[:].rearrange("r (j pl b) -> r j pl b", j=4, pl=4, b=32)
            dst = O[:].rearrange(
                "r (b g j pl) -> r g j pl b", b=32, g=8, j=4, pl=4)[:, g]
            copy(dst, src, kind="scatter")

        nc.sync.dma_start(out[128 * i:128 * (i + 1), :], O[:])


def _emit_kernel_v2(ctx, tc, out, x, amats, bmats, ident):
    """f32r weights-stationary variant: super-tiles of 256 rows, stage
    matmuls lhsT=matrix rhs=data at N=256 (f32r streams 1 cyc/row vs 4 for
    fp32), data kept feature-major between stages, f32r transposes (1.5
    cyc/row) for all shuffles after the first exact fp32 transpose."""
    import concourse.mybir as mybir

    nc = tc.nc
    f32 = mybir.dt.float32
    f32r = mybir.dt.float32r

    consts = ctx.enter_context(tc.tile_pool(name="consts", bufs=1))
    mstage = ctx.enter_context(tc.tile_pool(name="mstage", bufs=1))
    xin = ctx.enter_context(tc.tile_pool(name="xin", bufs=2))
    xTrp = ctx.enter_context(tc.tile_pool(name="xTrp", bufs=1))
    ypool = ctx.enter_context(tc.tile_pool(name="ypool", bufs=4))
    zpool = ctx.enter_context(tc.tile_pool(name="zpool", bufs=4))
    wpool = ctx.enter_context(tc.tile_pool(name="wpool", bufs=4))
    Ypool = ctx.enter_context(tc.tile_pool(name="Ypool", bufs=2))
    Opool = ctx.enter_context(tc.tile_pool(name="Opool", bufs=2))
    psT = ctx.enter_context(tc.tile_pool(name="psT", bufs=3, space="PSUM"))
    psM = ctx.enter_context(tc.tile_pool(name="psM", bufs=3, space="PSUM"))

    # constants: round matrices + identity to f32r on device
    amr = consts.tile([128, DIM], f32r, tag="amr")
    bmr = consts.tile([128, DIM], f32r, tag="bmr")
    idt = consts.tile([128, 128], f32, tag="idt")
    idtr = consts.tile([128, 128], f32r, tag="idtr")
    nc.sync.dma_start(idt[:], ident[:])
    nc.vector.tensor_copy(idtr[:], idt[:])
    am_st = mstage.tile([128, DIM], f32, tag="mst")
    for c in range(4):
        nc.sync.dma_start(am_st[:, 1024 * c:1024 * (c + 1)],
                          amats[:, 1024 * c:1024 * (c + 1)])
    for c in range(4):
        eng = nc.vector.tensor_copy if c % 2 else nc.scalar.copy
        eng(amr[:, 1024 * c:1024 * (c + 1)],
            am_st[:, 1024 * c:1024 * (c + 1)])
    bm_st = mstage.tile([128, DIM], f32, tag="mst")
    for c in range(4):
        nc.sync.dma_start(bm_st[:, 1024 * c:1024 * (c + 1)],
                          bmats[:, 1024 * c:1024 * (c + 1)])
    for c in range(4):
        eng = nc.vector.tensor_copy if c % 2 else nc.scalar.copy
        eng(bmr[:, 1024 * c:1024 * (c + 1)],
            bm_st[:, 1024 * c:1024 * (c + 1)])

    load = {"dve": 0.0, "act": 0.0}
    cost = {("dve", "plain"): 685, ("dve", "scatter"): 700,
            ("act", "plain"): 570, ("act", "scatter"): 1127}

    def copy(dst, src, kind="plain"):
        eng = min(("dve", "act"), key=lambda e: load[e] + cost[(e, kind)])
        load[eng] += cost[(eng, kind)]
        (nc.vector.tensor_copy if eng == "dve" else nc.scalar.copy)(dst, src)

    NST = NT // 2            # super-tiles of 256 rows
    for s in range(NST):
        # ---- T1: exact fp32 transposes x -> xTrBig [f', (b, c r-chunk)] f32r
        xTr = xTrp.tile([128, 32 * 256], f32r, tag="xTr")
        for c in range(2):
            xt = xin.tile([128, DIM], f32, tag="xt")
            nc.sync.dma_start(
                xt[:], x[256 * s + 128 * c:256 * s + 128 * (c + 1), :])
            for g in range(8):
                pt = psT.tile([128, 512], f32, tag="psT")
                for j in range(4):
                    b = 4 * g + j
                    nc.tensor.transpose(
                        pt[:, 128 * j:128 * (j + 1)],
                        xt[:, 128 * b:128 * (b + 1)], idt[:])
                # dest: col 256*(4g+j) + 128c + q
                dst = xTr[:].rearrange(
                    "f (bb cc q) -> f cc bb q", bb=32, cc=2, q=128)
                dst = dst[:, c, 4 * g:4 * g + 4]        # [128, 4, 128]
                src = pt[:].rearrange("f (j q) -> f j q", j=4, q=128)
                copy(dst, src)
        # ---- M1 + T2 interleaved per 4-block group: stage A f32r N=256,
        # then f32r transposes y -> Y_c rows-major (b-major contiguous)
        Ys = [Ypool.tile([128, DIM], f32r, tag="Y", name=f"Yc{c}")
              for c in range(2)]
        for g in range(8):
            ySBs = []
            for jj in range(2):
                q = 2 * g + jj
                pm = psM.tile([128, 512], f32, tag="psM")
                for j in range(2):
                    b = 2 * q + j
                    nc.tensor.matmul(
                        pm[:, 256 * j:256 * (j + 1)],
                        amr[:, 128 * b:128 * (b + 1)],
                        xTr[:, 256 * b:256 * (b + 1)],
                        start=True, stop=True)
                ySB = ypool.tile([128, 512], f32r, tag="ySB")
                copy(ySB[:], pm[:])
                ySBs.append(ySB)
            for c in range(2):
                pt = psT.tile([128, 512], f32r, tag="psT")
                for j in range(4):
                    b = 4 * g + j
                    jj, bb = b // 2 - 2 * g, b % 2
                    nc.tensor.transpose(
                        pt[:, 128 * j:128 * (j + 1)],
                        ySBs[jj][:, 256 * bb + 128 * c:256 * bb + 128 * (c + 1)],
                        idtr[:])
                # scatter into f~ order: dest = (p//4)*128 + (p%4)*32 + (4g+j)
                srcv = pt[:].rearrange(
                    "r (j tt pl) -> r j tt pl", j=4, tt=32, pl=4)
                dstv = Ys[c][:].rearrange(
                    "r (tt pl gg j) -> r gg j tt pl",
                    tt=32, pl=4, gg=8, j=4)[:, g]
                copy(dstv, srcv, kind="scatter")
        # ---- T3 + M2 + T4 interleaved per 4-tile group
        Os = [Opool.tile([128, DIM], f32, tag="O", name=f"Oc{c}")
              for c in range(2)]
        for g in range(8):
            wSBs = []
            for jj in range(2):
                q = 2 * g + jj
                pt = psT.tile([128, 512], f32r, tag="psT")
                for j in range(2):
                    t = 2 * q + j
                    for c in range(2):
                        nc.tensor.transpose(
                            pt[:, 256 * j + 128 * c:256 * j + 128 * (c + 1)],
                            Ys[c][:, 128 * t:128 * (t + 1)], idtr[:])
                zr = zpool.tile([128, 512], f32r, tag="zr")
                copy(zr[:], pt[:])
                pw = psM.tile([128, 512], f32, tag="psM")
                for j in range(2):
                    t = 2 * q + j
                    nc.tensor.matmul(
                        pw[:, 256 * j:256 * (j + 1)],
                        bmr[:, 128 * t:128 * (t + 1)],
                        zr[:, 256 * j:256 * (j + 1)],
                        start=True, stop=True)
                wSB = wpool.tile([128, 512], f32r, tag="wSB")
                copy(wSB[:], pw[:])
                wSBs.append(wSB)
            for c in range(2):
                pt = psT.tile([128, 512], f32r, tag="psT")
                for j in range(4):
                    t = 4 * g + j
                    jj, tt = t // 2 - 2 * g, t % 2
                    nc.tensor.transpose(
                        pt[:, 128 * j:128 * (j + 1)],
                        wSBs[jj][:, 256 * tt + 128 * c:256 * tt + 128 * (c + 1)],
                        idtr[:])
                # dest f = b*128 + 16g + 4j + pl ; src col = j*128 + pl*32 + b
                src = pt[:].rearrange("r (j pl b) -> r b j pl", j=4, pl=4, b=32)
                dst = Os[c][:].rearrange(
                    "r (b gg j pl) -> r gg b j pl", b=32, gg=8, j=4, pl=4)[:, g]
                copy(dst, src, kind="scatter")
        for c in range(2):
            nc.sync.dma_start(
                out[256 * s + 128 * c:256 * s + 128 * (c + 1), :], Os[c][:])


def _emit_kernel_v3(ctx, tc, out, x, amats, bmats, ident16):
    """All-fp16 pipeline with fp16 DRAM I/O (host converts x to fp16 and the
    fp16 result back to fp32 - halves both DMA directions). Transposes and
    matmuls all run 1 cyc/col. Y kept in natural feature order (plain PSUM
    copies); the stage-B permutation is absorbed by single-stride transpose
    APs (Y[:, c0::32]) plus the host-interleaved bmI matrix."""
    import concourse.mybir as mybir

    nc = tc.nc
    f32 = mybir.dt.float32
    f16 = mybir.dt.float16

    consts = ctx.enter_context(tc.tile_pool(name="consts", bufs=1))
    xin = ctx.enter_context(tc.tile_pool(name="xin", bufs=3))
    ypool = ctx.enter_context(tc.tile_pool(name="ypool", bufs=2))
    opool = ctx.enter_context(tc.tile_pool(name="opool", bufs=2))
    sbst = ctx.enter_context(tc.tile_pool(name="sbst", bufs=8))
    psT16 = ctx.enter_context(tc.tile_pool(name="psT16", bufs=3, space="PSUM"))
    psM = ctx.enter_context(tc.tile_pool(name="psM", bufs=5, space="PSUM"))

    am = consts.tile([128, DIM], f16, tag="amats")
    bm = consts.tile([128, DIM], f16, tag="bmats")
    idt16 = consts.tile([128, 128], f16, tag="ident16")
    nc.sync.dma_start(idt16[:], ident16[:])

    # Greedy least-loaded engine assignment for copies, with per-kind costs
    # (ns, [128,512] tiles) from the TRN2 cost model.
    load = {"dve": 0.0, "act": 0.0}
    cost = {
        ("dve", "psum16"): 392, ("act", "psum16"): 570,
        ("dve", "psum32"): 658, ("act", "psum32"): 570,
        ("dve", "scatter32"): 700, ("act", "scatter32"): 1127,
    }
    eng_fn = {"dve": nc.vector.tensor_copy, "act": nc.scalar.copy}

    def copy(dst, src, kind):
        eng = min(("dve", "act"), key=lambda e: load[e] + cost[(e, kind)])
        load[eng] += cost[(eng, kind)]
        eng_fn[eng](dst, src)

    for i in range(NT):
        xt = xin.tile([128, DIM], f16, tag="xt")
        if i == 0:
            # interleave first x tile with the constants so PE starts early
            for c in range(8):
                nc.sync.dma_start(xt[:, 512 * c:512 * (c + 1)],
                                  x[0:128, 512 * c:512 * (c + 1)])
                nc.sync.dma_start(am[:, 512 * c:512 * (c + 1)],
                                  amats[:, 512 * c:512 * (c + 1)])
                nc.sync.dma_start(bm[:, 512 * c:512 * (c + 1)],
                                  bmats[:, 512 * c:512 * (c + 1)])
        else:
            nc.sync.dma_start(xt[:], x[128 * i:128 * (i + 1), :])

        Y = ypool.tile([128, DIM], f16, tag="Y")
        for g in range(8):           # stage A: groups of 4 feature blocks
            pt = psT16.tile([128, 512], f16, tag="ptT16")
            for j in range(4):
                b = 4 * g + j
                nc.tensor.transpose(
                    pt[:, 128 * j:128 * (j + 1)],
                    xt[:, 128 * b:128 * (b + 1)], idt16[:])
            xT4 = sbst.tile([128, 512], f16, tag="xT4")
            copy(xT4[:], pt[:], "psum16")
            pm = psM.tile([128, 512], f32, tag="pmM")
            for j in range(4):
                b = 4 * g + j
                nc.tensor.matmul(
                    pm[:, 128 * j:128 * (j + 1)],
                    xT4[:, 128 * j:128 * (j + 1)],
                    am[:, 128 * b:128 * (b + 1)],
                    start=True, stop=True)
            # col c of pm is feature f = 512g + c: plain contiguous copy
            copy(Y[:, 512 * g:512 * (g + 1)], pm[:], "psum32")

        O = opool.tile([128, DIM], f16, tag="O")
        # stage-B tile c0 reads Y cols {c0 + 32k} (single-stride AP): col k
        # holds (t, i) = (k//4, k%4) i.e. feature f = 128t + (c0 + 32i); the
        # transpose puts it at partition q~ = 4t + i, matching bmI's layout.
        Yv = Y[:].rearrange("r (k s) -> r s k", k=128, s=32)
        for g in range(8):           # stage B: groups of 4 c0-tiles
            pt = psT16.tile([128, 512], f16, tag="ptT16")
            for j in range(4):
                c0 = 4 * g + j
                nc.tensor.transpose(
                    pt[:, 128 * j:128 * (j + 1)], Yv[:, c0], idt16[:])
            z4 = sbst.tile([128, 512], f16, tag="xT4")
            copy(z4[:], pt[:], "psum16")
            pm = psM.tile([128, 512], f32, tag="pmM")
            for j in range(4):
                c0 = 4 * g + j
                nc.tensor.matmul(
                    pm[:, 128 * j:128 * (j + 1)],
                    z4[:, 128 * j:128 * (j + 1)],
                    bm[:, 128 * c0:128 * (c0 + 1)],
                    start=True, stop=True)
            # src col = 128j + 4t' + i  ->  dest f = 128t' + 32i + 4g + j
            src = pm[:].rearrange("r (j t i) -> r j t i", j=4, t=32, i=4)
            dst = O[:].rearrange(
                "r (t i gg j) -> r gg j t i", t=32, i=4, gg=8, j=4)[:, g]
            copy(dst, src, "scatter32")

        nc.sync.dma_start(out[128 * i:128 * (i + 1), :], O[:])


def _hoist_matmul_waits(nc):
    """Walrus's fp32/transpose matmul (self-loading LDWEIGHTS) accepts fewer
    sync waits than Tile may assign. Hoist multi-waits onto a PE NoOp inserted
    just before the matmul — same engine queue, so ordering is identical."""
    import concourse.mybir as mybir

    n_hoisted = 0
    for blk in nc.m.functions[0].blocks:
        il = blk.instructions
        i = 0
        while i < len(il):
            inst = il[i]
            si = inst.sync_info
            if (si is not None and len(si.on_wait) > 1
                    and not isinstance(inst, mybir.InstNoOp)):
                waits = list(si.on_wait)
                # keep the last wait on the matmul; one NoOp per extra wait
                # (cayman instructions carry at most one sem-wait each)
                for k, w in enumerate(waits[:-1]):
                    nop = mybir.InstNoOp(
                        name=f"{inst.name}_hw{k}", engine=inst.engine,
                        bass_nofuse=True)
                    nop.sync_info = mybir.SyncInfo(on_wait=[w], on_update=[])
                    nc.register_instruction(nop, overwrite=True)
                    il.insert(i, nop)
                    i += 1
                    n_hoisted += 1
                inst.sync_info = mybir.SyncInfo(
                    on_wait=[waits[-1]], on_update=list(si.on_update))
            i += 1
    return n_hoisted


_CACHED = {}
VARIANT = "v3"   # "v1" fused-fp32 | "v2" f32r weights-stationary | "v3" fp16


def _build_bass(variant=None):
    variant = variant or VARIANT
    if variant in _CACHED:
        return _CACHED[variant]
    from contextlib import ExitStack
    import concourse.bass as bass
    import concourse.tile as tile
    import concourse.mybir as mybir

    f32 = mybir.dt.float32
    f16 = mybir.dt.float16
    cdt = f16 if variant == "v3" else f32
    nc = bass.Bass("TRN2", target_bir_lowering=False, debug=False,
                   num_devices=NCORES)
    x = nc.dram_tensor("x", [RPC, DIM], cdt, kind="ExternalInput").ap()
    amats = nc.dram_tensor("amats", [128, DIM], cdt, kind="ExternalInput").ap()
    bmats = nc.dram_tensor("bmats", [128, DIM], cdt, kind="ExternalInput").ap()
    ident = nc.dram_tensor("ident", [128, 128], cdt, kind="ExternalInput").ap()
    out = nc.dram_tensor("out", [RPC, DIM], cdt, kind="ExternalOutput").ap()

    with tile.TileContext(nc) as tc:
        with ExitStack() as ctx:
            if variant == "v3":
                _emit_kernel_v3(ctx, tc, out, x, amats, bmats, ident)
            else:
                emit = {"v1": _emit_kernel, "v2": _emit_kernel_v2}[variant]
                emit(ctx, tc, out, x, amats, bmats, ident)

    _hoist_matmul_waits(nc)
    _CACHED[variant] = nc
    return nc


def make_in_maps(x, angles):
    cdt = np.float16 if VARIANT == "v3" else np.float32
    x = np.ascontiguousarray(np.asarray(x).astype(cdt))
    if VARIANT == "v3":
        amats, _ = _build_mats(angles)
        bmats = _build_bmI(angles)
    else:
        amats, bmats = _build_mats(angles)
    amats = amats.astype(cdt)
    bmats = bmats.astype(cdt)
    ident = np.eye(128, dtype=cdt)
    base = {"amats": amats, "bmats": bmats, "ident": ident}
    return [
        {"x": x[c * RPC:(c + 1) * RPC], **base}
        for c in range(NCORES)
    ]


def run_on_hw(x, angles, trace=False, trace_kwargs=None):
    from concourse.bass_utils import run_bass_kernel_spmd
    nc = _build_bass()
    in_maps = make_in_maps(x, angles)
    res = run_bass_kernel_spmd(
        nc, in_maps, core_ids=list(range(NCORES)), trace=trace,
        **(trace_kwargs or {}))
    out = np.concatenate(
        [np.asarray(res.results[c]["out"], np.float32) for c in range(NCORES)],
        axis=0)
    return out, res


def kernel(x, angles):
    last_err = None
    for attempt in range(3):
        try:
            out, _ = run_on_hw(x, angles, trace=False)
            return np.ascontiguousarray(out.astype(np.float32))
        except Exception as e:  # transient NRT/device errors: retry
            last_err = e
            import time
            time.sleep(5)
    raise last_err



# revision 19
# speedup vs baseline: 1.7156x; 1.7156x over previous
"""Butterfly (Givens) rotation network on TRN2, 8 NeuronCores.

Algorithm
---------
x: (8192, 4096) f32. 12 butterfly layers; layer l rotates pairs of features
differing in bit l of the feature index. Split into two linear stages:

  Stage A = layers 0-6: features mix only within 128-wide blocks b (bits 0-6)
            -> per-block 128x128 matrix A_b.
  Stage B = layers 7-11: features mix only across blocks at fixed within-block
            position p (bits 7-11) -> per-p 32x32 matrix B_p; grouping 4
            consecutive p per 128-partition tile gives block-diag 128x128.

Per 128-row tile (rows on partitions), all on the TensorEngine:
  pass1: per block b: PE-transpose x_b -> xT_b [f',r] (PSUM->SBUF copy),
         MM out[r,fo] = sum_f' xT_b[f',r] * A_bT[f',fo]  (lhsT=xT_b, rhs=A_bT)
         scatter-copy PSUM->SBUF into Y with f~ = (p//4)*128 + (p%4)*32 + b.
  pass2: per f~-tile t: PE-transpose Y_t -> z [f~',r],
         MM out[r,n] = sum z[f~',r] * BDT_t[f~',n], scatter-copy to natural
         feature order, DMA out.

Sharding: data-parallel over rows, 1024 rows/core; matrices replicated.
"""

import math
import numpy as np

DIM = 4096
NL = 12
NB = 32          # 128-wide feature blocks
ROWS = 8192
NCORES = 8
RPC = ROWS // NCORES     # rows per core
NT = RPC // 128          # 128-row tiles per core


# ---------------------------------------------------------------- host math

def _butterfly_np(x, angles):
    """float64 numpy copy of the reference butterfly."""
    x = np.asarray(x, np.float64)
    angles = np.asarray(angles, np.float64)
    B, d = x.shape
    for l in range(angles.shape[0]):
        stride = 2 ** l
        nblocks = d // (2 * stride)
        xr = x.reshape(B, nblocks, 2, stride)
        c = np.cos(angles[l]).reshape(nblocks, stride)
        s = np.sin(angles[l]).reshape(nblocks, stride)
        xi = xr[:, :, 0, :].copy()
        xj = xr[:, :, 1, :].copy()
        x = np.stack([c * xi + s * xj, -s * xi + c * xj], axis=2).reshape(B, d)
    return x


def _build_mats(angles):
    """Returns (amats, bmats) each [128, 4096] f32 in SBUF-ready layout."""
    angles = np.asarray(angles, np.float64)
    amats = np.zeros((128, DIM), np.float64)
    for b in range(NB):
        # A_bT[f_in, f_out]: butterfly of identity rows = F^T for this block
        amats[:, 128 * b:128 * b + 128] = _butterfly_np(
            np.eye(128), angles[0:7, 64 * b:64 * b + 64])
    bmats = np.zeros((128, DIM), np.float64)
    for t in range(32):
        for pl in range(4):
            p = 4 * t + pl
            BpT = _butterfly_np(np.eye(32), angles[7:12, p::128])
            bmats[32 * pl:32 * pl + 32, 128 * t + 32 * pl:128 * t + 32 * pl + 32] = BpT
    return amats.astype(np.float32), bmats.astype(np.float32)


def _build_bmI(angles):
    """Stage-B matrix for v3's tiling: tile c0 holds p in {c0, c0+32, c0+64,
    c0+96} at partitions q~ = 4t + i (t = block, i = p//32), an interleaved
    block-diagonal of the four 32x32 B_p matrices."""
    angles = np.asarray(angles, np.float64)
    bmI = np.zeros((128, DIM), np.float64)
    for c0 in range(32):
        for i in range(4):
            p = c0 + 32 * i
            BpT = _butterfly_np(np.eye(32), angles[7:12, p::128])
            rows = 4 * np.arange(32) + i
            cols = 128 * c0 + 4 * np.arange(32) + i
            bmI[np.ix_(rows, cols)] = BpT
    return bmI.astype(np.float32)


# ---------------------------------------------------------------- bass kernel

def _emit_kernel(ctx, tc, out, x, amats, bmats, ident):
    import concourse.bass as bass
    import concourse.mybir as mybir

    nc = tc.nc
    f32 = mybir.dt.float32

    consts = ctx.enter_context(tc.tile_pool(name="consts", bufs=1))
    xin = ctx.enter_context(tc.tile_pool(name="xin", bufs=3))
    ystage = ctx.enter_context(tc.tile_pool(name="ystage", bufs=3))
    ostage = ctx.enter_context(tc.tile_pool(name="ostage", bufs=3))
    sbst = ctx.enter_context(tc.tile_pool(name="sbst", bufs=6))
    psA = ctx.enter_context(tc.tile_pool(name="psA", bufs=4, space="PSUM"))
    psB = ctx.enter_context(tc.tile_pool(name="psB", bufs=4, space="PSUM"))

    am = consts.tile([128, DIM], f32, tag="amats")
    bm = consts.tile([128, DIM], f32, tag="bmats")
    idt = consts.tile([128, 128], f32, tag="ident")
    nc.sync.dma_start(idt[:], ident[:])

    # Greedy least-loaded assignment of PSUM->SBUF copies to DVE/ACT,
    # using measured per-copy costs (ns) for [128,512] fp32 from PSUM.
    load = {"dve": 0.0, "act": 0.0}
    cost = {("dve", "plain"): 685, ("dve", "scatter"): 700,
            ("act", "plain"): 570, ("act", "scatter"): 1127}

    def copy(dst, src, kind="plain"):
        eng = min(("dve", "act"), key=lambda e: load[e] + cost[(e, kind)])
        load[eng] += cost[(eng, kind)]
        (nc.vector.tensor_copy if eng == "dve" else nc.scalar.copy)(dst, src)

    for i in range(NT):
        xt = xin.tile([128, DIM], f32, tag="xt")
        if i == 0:
            # first tile: fine-grained x/amats chunk interleave so the very
            # first transposes and stage-A matmuls start as early as possible
            for c in range(8):
                nc.sync.dma_start(xt[:, 512 * c:512 * (c + 1)],
                                  x[0:128, 512 * c:512 * (c + 1)])
                nc.sync.dma_start(am[:, 512 * c:512 * (c + 1)],
                                  amats[:, 512 * c:512 * (c + 1)])
        else:
            nc.sync.dma_start(xt[:], x[128 * i:128 * (i + 1), :])
        Y = ystage.tile([128, DIM], f32, tag="Y")

        for g in range(8):           # groups of 4 feature blocks
            pt = psA.tile([128, 512], f32, tag="ptA")
            for j in range(4):
                b = 4 * g + j
                nc.tensor.transpose(
                    pt[:, 128 * j:128 * (j + 1)],
                    xt[:, 128 * b:128 * (b + 1)], idt[:])
            xT4 = sbst.tile([128, 512], f32, tag="xT4")
            copy(xT4[:], pt[:])
            pm = psB.tile([128, 512], f32, tag="pmA")
            for j in range(4):
                b = 4 * g + j
                nc.tensor.matmul(
                    pm[:, 128 * j:128 * (j + 1)],
                    xT4[:, 128 * j:128 * (j + 1)],
                    am[:, 128 * b:128 * (b + 1)],
                    start=True, stop=True)
            # scatter into Y: dest f~ = t*128 + pl*32 + (4g+j), src = j*128 + 4t + pl
            src = pm[:].rearrange("r (j t pl) -> r j t pl", j=4, t=32, pl=4)
            dst = Y[:].rearrange(
                "r (t pl g j) -> r g j t pl", t=32, pl=4, g=8, j=4)[:, g]
            copy(dst, src, kind="scatter")

        if i == 0:
            for c in range(8):
                nc.sync.dma_start(bm[:, 512 * c:512 * (c + 1)],
                                  bmats[:, 512 * c:512 * (c + 1)])
        O = ostage.tile([128, DIM], f32, tag="O")
        for g in range(8):           # groups of 4 f~ tiles
            pt = psA.tile([128, 512], f32, tag="ptA")
            for j in range(4):
                t = 4 * g + j
                nc.tensor.transpose(
                    pt[:, 128 * j:128 * (j + 1)],
                    Y[:, 128 * t:128 * (t + 1)], idt[:])
            z4 = sbst.tile([128, 512], f32, tag="xT4")
            copy(z4[:], pt[:])
            pm = psB.tile([128, 512], f32, tag="pmA")
            for j in range(4):
                t = 4 * g + j
                nc.tensor.matmul(
                    pm[:, 128 * j:128 * (j + 1)],
                    z4[:, 128 * j:128 * (j + 1)],
                    bm[:, 128 * t:128 * (t + 1)],
                    start=True, stop=True)
            # scatter to natural order: dest f = b*128 + 4t + pl = b*128 + 16g + 4j + pl
            src = pm[:].rearrange("r (j pl b) -> r j pl b", j=4, pl=4, b=32)
            dst = O[:].rearrange(
                "r (b g j pl) -> r g j pl b", b=32, g=8, j=4, pl=4)[:, g]
            copy(dst, src, kind="scatter")

        nc.sync.dma_start(out[128 * i:128 * (i + 1), :], O[:])


def _emit_kernel_v2(ctx, tc, out, x, amats, bmats, ident):
    """f32r weights-stationary variant: super-tiles of 256 rows, stage
    matmuls lhsT=matrix rhs=data at N=256 (f32r streams 1 cyc/row vs 4 for
    fp32), data kept feature-major between stages, f32r transposes (1.5
    cyc/row) for all shuffles after the first exact fp32 transpose."""
    import concourse.mybir as mybir

    nc = tc.nc
    f32 = mybir.dt.float32
    f32r = mybir.dt.float32r

    consts = ctx.enter_context(tc.tile_pool(name="consts", bufs=1))
    mstage = ctx.enter_context(tc.tile_pool(name="mstage", bufs=1))
    xin = ctx.enter_context(tc.tile_pool(name="xin", bufs=2))
    xTrp = ctx.enter_context(tc.tile_pool(name="xTrp", bufs=1))
    ypool = ctx.enter_context(tc.tile_pool(name="ypool", bufs=4))
    zpool = ctx.enter_context(tc.tile_pool(name="zpool", bufs=4))
    wpool = ctx.enter_context(tc.tile_pool(name="wpool", bufs=4))
    Ypool = ctx.enter_context(tc.tile_pool(name="Ypool", bufs=2))
    Opool = ctx.enter_context(tc.tile_pool(name="Opool", bufs=2))
    psT = ctx.enter_context(tc.tile_pool(name="psT", bufs=3, space="PSUM"))
    psM = ctx.enter_context(tc.tile_pool(name="psM", bufs=3, space="PSUM"))

    # constants: round matrices + identity to f32r on device
    amr = consts.tile([128, DIM], f32r, tag="amr")
    bmr = consts.tile([128, DIM], f32r, tag="bmr")
    idt = consts.tile([128, 128], f32, tag="idt")
    idtr = consts.tile([128, 128], f32r, tag="idtr")
    nc.sync.dma_start(idt[:], ident[:])
    nc.vector.tensor_copy(idtr[:], idt[:])
    am_st = mstage.tile([128, DIM], f32, tag="mst")
    for c in range(4):
        nc.sync.dma_start(am_st[:, 1024 * c:1024 * (c + 1)],
                          amats[:, 1024 * c:1024 * (c + 1)])
    for c in range(4):
        eng = nc.vector.tensor_copy if c % 2 else nc.scalar.copy
        eng(amr[:, 1024 * c:1024 * (c + 1)],
            am_st[:, 1024 * c:1024 * (c + 1)])
    bm_st = mstage.tile([128, DIM], f32, tag="mst")
    for c in range(4):
        nc.sync.dma_start(bm_st[:, 1024 * c:1024 * (c + 1)],
                          bmats[:, 1024 * c:1024 * (c + 1)])
    for c in range(4):
        eng = nc.vector.tensor_copy if c % 2 else nc.scalar.copy
        eng(bmr[:, 1024 * c:1024 * (c + 1)],
            bm_st[:, 1024 * c:1024 * (c + 1)])

    load = {"dve": 0.0, "act": 0.0}
    cost = {("dve", "plain"): 685, ("dve", "scatter"): 700,
            ("act", "plain"): 570, ("act", "scatter"): 1127}

    def copy(dst, src, kind="plain"):
        eng = min(("dve", "act"), key=lambda e: load[e] + cost[(e, kind)])
        load[eng] += cost[(eng, kind)]
        (nc.vector.tensor_copy if eng == "dve" else nc.scalar.copy)(dst, src)

    NST = NT // 2            # super-tiles of 256 rows
    for s in range(NST):
        # ---- T1: exact fp32 transposes x -> xTrBig [f', (b, c r-chunk)] f32r
        xTr = xTrp.tile([128, 32 * 256], f32r, tag="xTr")
        for c in range(2):
            xt = xin.tile([128, DIM], f32, tag="xt")
            nc.sync.dma_start(
                xt[:], x[256 * s + 128 * c:256 * s + 128 * (c + 1), :])
            for g in range(8):
                pt = psT.tile([128, 512], f32, tag="psT")
                for j in range(4):
                    b = 4 * g + j
                    nc.tensor.transpose(
                        pt[:, 128 * j:128 * (j + 1)],
                        xt[:, 128 * b:128 * (b + 1)], idt[:])
                # dest: col 256*(4g+j) + 128c + q
                dst = xTr[:].rearrange(
                    "f (bb cc q) -> f cc bb q", bb=32, cc=2, q=128)
                dst = dst[:, c, 4 * g:4 * g + 4]        # [128, 4, 128]
                src = pt[:].rearrange("f (j q) -> f j q", j=4, q=128)
                copy(dst, src)
        # ---- M1 + T2 interleaved per 4-block group: stage A f32r N=256,
        # then f32r transposes y -> Y_c rows-major (b-major contiguous)
        Ys = [Ypool.tile([128, DIM], f32r, tag="Y", name=f"Yc{c}")
              for c in range(2)]
        for g in range(8):
            ySBs = []
            for jj in range(2):
                q = 2 * g + jj
                pm = psM.tile([128, 512], f32, tag="psM")
                for j in range(2):
                    b = 2 * q + j
                    nc.tensor.matmul(
                        pm[:, 256 * j:256 * (j + 1)],
                        amr[:, 128 * b:128 * (b + 1)],
                        xTr[:, 256 * b:256 * (b + 1)],
                        start=True, stop=True)
                ySB = ypool.tile([128, 512], f32r, tag="ySB")
                copy(ySB[:], pm[:])
                ySBs.append(ySB)
            for c in range(2):
                pt = psT.tile([128, 512], f32r, tag="psT")
                for j in range(4):
                    b = 4 * g + j
                    jj, bb = b // 2 - 2 * g, b % 2
                    nc.tensor.transpose(
                        pt[:, 128 * j:128 * (j + 1)],
                        ySBs[jj][:, 256 * bb + 128 * c:256 * bb + 128 * (c + 1)],
                        idtr[:])
                # scatter into f~ order: dest = (p//4)*128 + (p%4)*32 + (4g+j)
                srcv = pt[:].rearrange(
                    "r (j tt pl) -> r j tt pl", j=4, tt=32, pl=4)
                dstv = Ys[c][:].rearrange(
                    "r (tt pl gg j) -> r gg j tt pl",
                    tt=32, pl=4, gg=8, j=4)[:, g]
                copy(dstv, srcv, kind="scatter")
        # ---- T3 + M2 + T4 interleaved per 4-tile group
        Os = [Opool.tile([128, DIM], f32, tag="O", name=f"Oc{c}")
              for c in range(2)]
        for g in range(8):
            wSBs = []
            for jj in range(2):
                q = 2 * g + jj
                pt = psT.tile([128, 512], f32r, tag="psT")
                for j in range(2):
                    t = 2 * q + j
                    for c in range(2):
                        nc.tensor.transpose(
                            pt[:, 256 * j + 128 * c:256 * j + 128 * (c + 1)],
                            Ys[c][:, 128 * t:128 * (t + 1)], idtr[:])
                zr = zpool.tile([128, 512], f32r, tag="zr")
                copy(zr[:], pt[:])
                pw = psM.tile([128, 512], f32, tag="psM")
                for j in range(2):
                    t = 2 * q + j
                    nc.tensor.matmul(
                        pw[:, 256 * j:256 * (j + 1)],
                        bmr[:, 128 * t:128 * (t + 1)],
                        zr[:, 256 * j:256 * (j + 1)],
                        start=True, stop=True)
                wSB = wpool.tile([128, 512], f32r, tag="wSB")
                copy(wSB[:], pw[:])
                wSBs.append(wSB)
            for c in range(2):
                pt = psT.tile([128, 512], f32r, tag="psT")
                for j in range(4):
                    t = 4 * g + j
                    jj, tt = t // 2 - 2 * g, t % 2
                    nc.tensor.transpose(
                        pt[:, 128 * j:128 * (j + 1)],
                        wSBs[jj][:, 256 * tt + 128 * c:256 * tt + 128 * (c + 1)],
                        idtr[:])
                # dest f = b*128 + 16g + 4j + pl ; src col = j*128 + pl*32 + b
                src = pt[:].rearrange("r (j pl b) -> r b j pl", j=4, pl=4, b=32)
                dst = Os[c][:].rearrange(
                    "r (b gg j pl) -> r gg b j pl", b=32, gg=8, j=4, pl=4)[:, g]
                copy(dst, src, kind="scatter")
        for c in range(2):
            nc.sync.dma_start(
                out[256 * s + 128 * c:256 * s + 128 * (c + 1), :], Os[c][:])


def _emit_kernel_v3(ctx, tc, out, x, amats, bmats, ident16):
    """All-fp16 pipeline with fp16 DRAM I/O (host converts x to fp16 and the
    fp16 result back to fp32 - halves both DMA directions). Transposes and
    matmuls all run 1 cyc/col. Y kept in natural feature order (plain PSUM
    copies); the stage-B permutation is absorbed by single-stride transpose
    APs (Y[:, c0::32]) plus the host-interleaved bmI matrix."""
    import concourse.mybir as mybir

    nc = tc.nc
    f32 = mybir.dt.float32
    f16 = mybir.dt.float16

    consts = ctx.enter_context(tc.tile_pool(name="consts", bufs=1))
    xin = ctx.enter_context(tc.tile_pool(name="xin", bufs=3))
    ypool = ctx.enter_context(tc.tile_pool(name="ypool", bufs=2))
    opool = ctx.enter_context(tc.tile_pool(name="opool", bufs=2))
    sbst = ctx.enter_context(tc.tile_pool(name="sbst", bufs=8))
    psT16 = ctx.enter_context(tc.tile_pool(name="psT16", bufs=3, space="PSUM"))
    psM = ctx.enter_context(tc.tile_pool(name="psM", bufs=5, space="PSUM"))

    am = consts.tile([128, DIM], f16, tag="amats")
    bm = consts.tile([128, DIM], f16, tag="bmats")
    idt16 = consts.tile([128, 128], f16, tag="ident16")
    nc.sync.dma_start(idt16[:], ident16[:])

    # Greedy least-loaded engine assignment for copies, with per-kind costs
    # (ns, [128,512] tiles) from the TRN2 cost model.
    load = {"dve": 0.0, "act": 0.0}
    cost = {
        ("dve", "psum16"): 392, ("act", "psum16"): 570,
        ("dve", "psum32"): 658, ("act", "psum32"): 570,
        ("dve", "scatter32"): 700, ("act", "scatter32"): 1127,
    }
    eng_fn = {"dve": nc.vector.tensor_copy, "act": nc.scalar.copy}

    def copy(dst, src, kind):
        eng = min(("dve", "act"), key=lambda e: load[e] + cost[(e, kind)])
        load[eng] += cost[(eng, kind)]
        eng_fn[eng](dst, src)

    for i in range(NT):
        xt = xin.tile([128, DIM], f16, tag="xt")
        if i == 0:
            # interleave first x tile with the constants so PE starts early
            for c in range(8):
                nc.sync.dma_start(xt[:, 512 * c:512 * (c + 1)],
                                  x[0:128, 512 * c:512 * (c + 1)])
                nc.sync.dma_start(am[:, 512 * c:512 * (c + 1)],
                                  amats[:, 512 * c:512 * (c + 1)])
                nc.sync.dma_start(bm[:, 512 * c:512 * (c + 1)],
                                  bmats[:, 512 * c:512 * (c + 1)])
        else:
            nc.sync.dma_start(xt[:], x[128 * i:128 * (i + 1), :])

        Y = ypool.tile([128, DIM], f16, tag="Y")
        for g in range(8):           # stage A: groups of 4 feature blocks
            pt = psT16.tile([128, 512], f16, tag="ptT16")
            for j in range(4):
                b = 4 * g + j
                nc.tensor.transpose(
                    pt[:, 128 * j:128 * (j + 1)],
                    xt[:, 128 * b:128 * (b + 1)], idt16[:])
            xT4 = sbst.tile([128, 512], f16, tag="xT4")
            copy(xT4[:], pt[:], "psum16")
            pm = psM.tile([128, 512], f32, tag="pmM")
            for j in range(4):
                b = 4 * g + j
                nc.tensor.matmul(
                    pm[:, 128 * j:128 * (j + 1)],
                    xT4[:, 128 * j:128 * (j + 1)],
                    am[:, 128 * b:128 * (b + 1)],
                    start=True, stop=True)
            # col c of pm is feature f = 512g + c: plain contiguous copy
            copy(Y[:, 512 * g:512 * (g + 1)], pm[:], "psum32")

        O = opool.tile([128, DIM], f16, tag="O")
        # stage-B tile c0 reads Y cols {c0 + 32k} (single-stride AP): col k
        # holds (t, i) = (k//4, k%4) i.e. feature f = 128t + (c0 + 32i); the
        # transpose puts it at partition q~ = 4t + i, matching bmI's layout.
        Yv = Y[:].rearrange("r (k s) -> r s k", k=128, s=32)
        for g in range(8):           # stage B: groups of 4 c0-tiles
            pt = psT16.tile([128, 512], f16, tag="ptT16")
            for j in range(4):
                c0 = 4 * g + j
                nc.tensor.transpose(
                    pt[:, 128 * j:128 * (j + 1)], Yv[:, c0], idt16[:])
            z4 = sbst.tile([128, 512], f16, tag="xT4")
            copy(z4[:], pt[:], "psum16")
            pm = psM.tile([128, 512], f32, tag="pmM")
            for j in range(4):
                c0 = 4 * g + j
                nc.tensor.matmul(
                    pm[:, 128 * j:128 * (j + 1)],
                    z4[:, 128 * j:128 * (j + 1)],
                    bm[:, 128 * c0:128 * (c0 + 1)],
                    start=True, stop=True)
            # src col = 128j + 4t' + i  ->  dest f = 128t' + 32i + 4g + j.
            # j innermost on both sides: dst runs are 4 contiguous fp16 (8B,
            # 4B-aligned); scattered single-fp16 writes are ~7x slower.
            src = pm[:].rearrange("r (j t i) -> r t i j", j=4, t=32, i=4)
            dst = O[:].rearrange(
                "r (t i gg j) -> r gg t i j", t=32, i=4, gg=8, j=4)[:, g]
            copy(dst, src, "scatter32")

        nc.sync.dma_start(out[128 * i:128 * (i + 1), :], O[:])


def _hoist_matmul_waits(nc):
    """Walrus's fp32/transpose matmul (self-loading LDWEIGHTS) accepts fewer
    sync waits than Tile may assign. Hoist multi-waits onto a PE NoOp inserted
    just before the matmul — same engine queue, so ordering is identical."""
    import concourse.mybir as mybir

    n_hoisted = 0
    for blk in nc.m.functions[0].blocks:
        il = blk.instructions
        i = 0
        while i < len(il):
            inst = il[i]
            si = inst.sync_info
            if (si is not None and len(si.on_wait) > 1
                    and not isinstance(inst, mybir.InstNoOp)):
                waits = list(si.on_wait)
                # keep the last wait on the matmul; one NoOp per extra wait
                # (cayman instructions carry at most one sem-wait each)
                for k, w in enumerate(waits[:-1]):
                    nop = mybir.InstNoOp(
                        name=f"{inst.name}_hw{k}", engine=inst.engine,
                        bass_nofuse=True)
                    nop.sync_info = mybir.SyncInfo(on_wait=[w], on_update=[])
                    nc.register_instruction(nop, overwrite=True)
                    il.insert(i, nop)
                    i += 1
                    n_hoisted += 1
                inst.sync_info = mybir.SyncInfo(
                    on_wait=[waits[-1]], on_update=list(si.on_update))
            i += 1
    return n_hoisted


_CACHED = {}
VARIANT = "v3"   # "v1" fused-fp32 | "v2" f32r weights-stationary | "v3" fp16


def _build_bass(variant=None):
    variant = variant or VARIANT
    if variant in _CACHED:
        return _CACHED[variant]
    from contextlib import ExitStack
    import concourse.bass as bass
    import concourse.tile as tile
    import concourse.mybir as mybir

    f32 = mybir.dt.float32
    f16 = mybir.dt.float16
    cdt = f16 if variant == "v3" else f32
    nc = bass.Bass("TRN2", target_bir_lowering=False, debug=False,
                   num_devices=NCORES)
    x = nc.dram_tensor("x", [RPC, DIM], cdt, kind="ExternalInput").ap()
    amats = nc.dram_tensor("amats", [128, DIM], cdt, kind="ExternalInput").ap()
    bmats = nc.dram_tensor("bmats", [128, DIM], cdt, kind="ExternalInput").ap()
    ident = nc.dram_tensor("ident", [128, 128], cdt, kind="ExternalInput").ap()
    out = nc.dram_tensor("out", [RPC, DIM], cdt, kind="ExternalOutput").ap()

    with tile.TileContext(nc) as tc:
        with ExitStack() as ctx:
            if variant == "v3":
                _emit_kernel_v3(ctx, tc, out, x, amats, bmats, ident)
            else:
                emit = {"v1": _emit_kernel, "v2": _emit_kernel_v2}[variant]
                emit(ctx, tc, out, x, amats, bmats, ident)

    _hoist_matmul_waits(nc)
    _CACHED[variant] = nc
    return nc


def make_in_maps(x, angles):
    cdt = np.float16 if VARIANT == "v3" else np.float32
    x = np.ascontiguousarray(np.asarray(x).astype(cdt))
    if VARIANT == "v3":
        amats, _ = _build_mats(angles)
        bmats = _build_bmI(angles)
    else:
        amats, bmats = _build_mats(angles)
    amats = amats.astype(cdt)
    bmats = bmats.astype(cdt)
    ident = np.eye(128, dtype=cdt)
    base = {"amats": amats, "bmats": bmats, "ident": ident}
    return [
        {"x": x[c * RPC:(c + 1) * RPC], **base}
        for c in range(NCORES)
    ]


def run_on_hw(x, angles, trace=False, trace_kwargs=None):
    from concourse.bass_utils import run_bass_kernel_spmd
    nc = _build_bass()
    in_maps = make_in_maps(x, angles)
    res = run_bass_kernel_spmd(
        nc, in_maps, core_ids=list(range(NCORES)), trace=trace,
        **(trace_kwargs or {}))
    out = np.concatenate(
        [np.asarray(res.results[c]["out"], np.float32) for c in range(NCORES)],
        axis=0)
    return out, res


def kernel(x, angles):
    last_err = None
    for attempt in range(3):
        try:
            out, _ = run_on_hw(x, angles, trace=False)
            return np.ascontiguousarray(out.astype(np.float32))
        except Exception as e:  # transient NRT/device errors: retry
            last_err = e
            import time
            time.sleep(5)
    raise last_err

